# revision 28
# baseline (speedup 1.0000x reference)
"""Trainium2 Bass kernel for GQA multi-head attention with RoPE.

Sharding: tensor-parallel over heads. Core c owns q-heads {c, c+8, c+16,
c+24} and kv-head c (q head h uses kv head h % 8). Each core computes its
QKV projection slice, RoPE, causal attention for its heads, and a partial
output projection (attn_out_local @ Wo[:, local].T) in fp16; the host sums
the 8 partial y's in fp32.

All matmul operands are fp16 (fp32/fp32r matmuls run in fp32_mode=HIGH at
less than half rate on TRN2). PSUM accumulation stays fp32.

Layouts (per core, per batch b):
  qT01/qT23 [128, T] roped q, two heads per tile (64 dims each)
  kvT       [128, T] rows 0:64 roped k, rows 64:128 v
  ktdup     [128, T] rows 64:128 = copy of roped k (odd heads read their
            stationary from partitions 64:128)
  scores    S^T [tk partitions, tq free]: exp(S^T) feeds P@V directly with
            v_aug = [v | ones] stationary; the ones row accumulates softmax
            row-sums in partition 64 of the PV accumulator.
  PV        accumulated per 512-token quarter ([65, 512] PSUM, 1 bank) so
            quarters drain (normalize into OT) while later k-blocks still
            accumulate the next quarter.
  y         [T*B, C] fp16 partial; host adds the 8 partials + bo.

Attention is processed one head per iteration (16 iterations of
(batch, head-pair, parity, tq-half)): per k-block the ACT engine does one
exp of [128, W] while the PE does W score columns + W PV columns, so the
two engines stay nearly balanced, and the PSUM budget (4 banks score
double-buffer + 2 banks PV quarters + 2 banks aux) leaves two slots for
QKV / output-projection work interleaved into the attention stream to keep
the PE dense (HAM re-throttles the PE clock to 1.2 GHz on idle gaps).
"""

import sys

sys.path.insert(0, "/opt/trn_rl_repo")

import numpy as np

import bass_rust
import concourse.bass as bass
import concourse.tile as tile
from concourse import mybir
from concourse.bass_utils import run_bass_kernel_spmd

B, T, C = 2, 2048, 2048
H, KVH, D = 32, 8, 64
NTOK = B * T                 # 4096
HPC = H // 8                 # 4 q heads per core
QL = HPC * D                 # 256 local q dims
KC = C // 128                # 16 contraction chunks
NJ = T // 128                # 16 k-blocks per batch
SCALE = float(D) ** -0.5

F32 = mybir.dt.float32
F16 = mybir.dt.float16
AF = mybir.ActivationFunctionType

_NC_CACHE = {}


def _split_waits(nc, limit=1):
    """Walrus in this toolchain allows only one sync-wait per instruction.

    Tile emits instructions with several sem waits (drain/barrier, phase
    boundaries). Hoist the excess onto same-engine NoOps inserted right
    before the instruction -- program order on the engine queue preserves
    the wait semantics.
    """
    ctr = 0
    for f in nc.m.functions:
        for blk in f.blocks:
            out = []
            changed = False
            for inst in list(blk.instructions):
                si = inst.sync_info
                if si is not None and len(si.on_wait) > limit:
                    waits = list(si.on_wait)
                    keep, excess = waits[:limit], waits[limit:]
                    for i in range(0, len(excess), limit):
                        ctr += 1
                        nop = mybir.InstNoOp(
                            name=f"I-wsplit-{ctr}", ins=[], outs=[]
                        )
                        nop.engine = inst.engine
                        nop.sync_info = bass_rust.SyncInfo(
                            on_wait=excess[i : i + limit], on_update=[]
                        )
                        out.append(nop)
                        changed = True
                    inst.sync_info = bass_rust.SyncInfo(
                        on_wait=keep, on_update=list(si.on_update)
                    )
                out.append(inst)
            if changed:
                blk.instructions = out
    return ctr


def build_nc():
    nc = bass.Bass(trn_type="TRN2")

    xT_d = nc.dram_tensor("xT", [C, NTOK], F16, kind="ExternalInput")
    wcat_d = nc.dram_tensor("wcatT", [C, 384], F16, kind="ExternalInput")
    bias_d = nc.dram_tensor("bqkv", [128, 3], F32, kind="ExternalInput")
    wo_d = nc.dram_tensor("woR", [QL, C], F16, kind="ExternalInput")
    cos_d = nc.dram_tensor("cos2", [128, T], F16, kind="ExternalInput")
    sin_d = nc.dram_tensor("sin2", [128, T], F16, kind="ExternalInput")
    pmat_d = nc.dram_tensor("pmat", [128, 128], F16, kind="ExternalInput")
    dmask_d = nc.dram_tensor("dmask", [128, 128], F16, kind="ExternalInput")
    ident_d = nc.dram_tensor("ident2", [128, 64], F16, kind="ExternalInput")
    vones_d = nc.dram_tensor("vones", [128, B * NJ], F16, kind="ExternalInput")
    y_d = nc.dram_tensor("y", [NTOK, C], F16, kind="ExternalOutput")

    xT_v = xT_d.rearrange("(k p) t -> p k t", p=128)

    with tile.TileContext(nc) as tc:
        with (
            tc.tile_pool(name="consts", bufs=1) as consts,
            tc.tile_pool(name="acts", bufs=1) as acts,
            tc.tile_pool(name="xs", bufs=2) as xs_pool,
            tc.tile_pool(name="tmp", bufs=2) as tmp_pool,
            tc.tile_pool(name="es", bufs=6) as es_pool,
            tc.tile_pool(name="lnr", bufs=4) as lnr_pool,
            tc.tile_pool(name="ibc", bufs=4) as ibc_pool,
            tc.tile_pool(name="ystage", bufs=3) as y_pool,
            tc.tile_pool(name="psS", bufs=2, space="PSUM") as psS,
            tc.tile_pool(name="psA", bufs=2, space="PSUM") as psA,
        ):
            # ---- constants (ident/cos first: the HAM warm-up needs them;
            # wcat k0-3 + the first x quarter unblock the first QKV chunk) ----
            ident_sb = consts.tile([128, 64], F16, tag="ident")
            nc.sync.dma_start(out=ident_sb, in_=ident_d[:, :])
            cos_sb = consts.tile([128, T], F16, tag="cos")
            nc.sync.dma_start(out=cos_sb, in_=cos_d[:, :])
            wcat_sb = consts.tile([128, KC, 384], F16, tag="wcat")
            for k in range(4):
                nc.sync.dma_start(
                    out=wcat_sb[:, k, :], in_=wcat_d[128 * k : 128 * (k + 1), :]
                )

            # HAM warm-up: dummy matmuls so the PE clock gate is at 8/8
            # (needs ~3.4us sustained busy) while the input DMAs stream
            wps = psA.tile([128, 512], F32, tag="aux")
            for i in range(32):
                nc.tensor.matmul(
                    wps[0:64, :],
                    ident_sb[0:64, :],
                    cos_sb[0:64, 0:512],
                    start=(i == 0),
                    stop=(i == 31),
                )
            wsink = consts.tile([64, 512], F16, tag="wsink")
            nc.vector.tensor_copy(wsink, wps[0:64, :])
            nc.gpsimd.dma_start(
                out=nc.dram_tensor("wsink_d", [64, 512], F16)[:, :],
                in_=wsink,
            )

            for k in range(4, KC):
                nc.sync.dma_start(
                    out=wcat_sb[:, k, :], in_=wcat_d[128 * k : 128 * (k + 1), :]
                )
            bias_sb = consts.tile([128, 3], F32, tag="bias")
            nc.scalar.dma_start(out=bias_sb, in_=bias_d[:, :])
            sin_sb = consts.tile([128, T], F16, tag="sin")
            nc.scalar.dma_start(out=sin_sb, in_=sin_d[:, :])
            pmat_sb = consts.tile([128, 128], F16, tag="pmat")
            nc.scalar.dma_start(out=pmat_sb, in_=pmat_d[:, :])
            dmask_sb = consts.tile([128, 128], F16, tag="dmask")
            nc.scalar.dma_start(out=dmask_sb, in_=dmask_d[:, :])
            wo_sb = consts.tile([128, 2, C], F16, tag="wo")
            for k in range(2):
                nc.gpsimd.dma_start(
                    out=wo_sb[:, k, :], in_=wo_d[128 * k : 128 * (k + 1), :]
                )

            def _mk(nm, shape, dt):
                return [
                    acts.tile(shape, dt, tag=f"{nm}_{b}", name=f"{nm}_{b}")
                    for b in range(B)
                ]

            qT01 = _mk("qT01", [128, T], F16)
            qT23 = _mk("qT23", [128, T], F16)
            kvT = _mk("kvT", [128, T], F16)
            ktdup = _mk("ktd", [128, T], F16)
            vaug = _mk("vaug", [128, NJ, 65], F16)
            OT0 = _mk("OT0", [128, T], F16)
            OT1 = _mk("OT1", [128, T], F16)
            for b in range(B):
                nc.gpsimd.dma_start(
                    out=vaug[b][:, :, 64:65],
                    in_=vones_d[:, NJ * b : NJ * (b + 1)].unsqueeze(2),
                )

            qkv_dst = [[qT01[b], qT23[b], kvT[b]] for b in range(B)]

            def emit_qkv_group(b2, gl):
                """QKV projection + RoPE + k-dup + v-transpose for 512 tokens.

                Returns emitter thunks (micro-pieces of <=1us of PE work) so
                the caller can weave them into the attention stream without
                stalling the exp chain.
                """
                base = 512 * gl
                gbase = T * b2 + base
                xts = []
                pss = {}

                def load_x():
                    # four DMAs so the first contraction chunk unblocks early
                    xt = xs_pool.tile([128, KC, 512], F16, tag="xs")
                    for kq in range(4):
                        nc.sync.dma_start(
                            out=xt[:, 4 * kq : 4 * kq + 4, :],
                            in_=xT_v[:, 4 * kq : 4 * kq + 4, gbase : gbase + 512],
                        )
                    xts.append(xt)

                def mm_chunk(m, kc):
                    # 4 of the 16 contraction steps of pass m
                    def go():
                        if m not in pss:
                            pss[m] = psA.tile(
                                [128, 512], F32, tag="aux",
                                name=f"qkvps_{b2}_{gl}_{m}",
                            )
                        ps = pss[m]
                        for k in range(4 * kc, 4 * kc + 4):
                            nc.tensor.matmul(
                                ps,
                                wcat_sb[:, k, 128 * m : 128 * (m + 1)],
                                xts[0][:, k, :],
                                start=(k == 0),
                                stop=(k == KC - 1),
                            )
                        if kc == 3:
                            nc.vector.tensor_scalar_add(
                                out=qkv_dst[b2][m][:, base : base + 512],
                                in0=ps,
                                scalar1=bias_sb[:, m : m + 1],
                            )
                    return go

                def first():
                    load_x()
                    mm_chunk(0, 0)()

                def rope(dst, rn):
                    def go():
                        rot = psA.tile([128, 512], F32, tag="aux")
                        nc.tensor.matmul(
                            rot[:rn, :],
                            pmat_sb[:rn, :rn],
                            dst[:rn, base : base + 512],
                            start=True,
                            stop=True,
                        )
                        tmp = tmp_pool.tile([128, 512], F16, tag="tmp")
                        nc.vector.tensor_mul(
                            tmp[:rn, :], rot[:rn, :],
                            sin_sb[:rn, base : base + 512],
                        )
                        nc.vector.tensor_mul(
                            dst[:rn, base : base + 512],
                            dst[:rn, base : base + 512],
                            cos_sb[:rn, base : base + 512],
                        )
                        nc.vector.tensor_add(
                            dst[:rn, base : base + 512],
                            dst[:rn, base : base + 512],
                            tmp[:rn, :],
                        )
                    return go

                def kv_tail():
                    # duplicate roped k into ktdup partitions 64:128
                    nc.gpsimd.dma_start(
                        out=ktdup[b2][64:128, base : base + 512],
                        in_=kvT[b2][0:64, base : base + 512],
                    )
                    # v transposes (v is not roped)
                    for jj in range(4):
                        jt = 4 * gl + jj
                        vps = psA.tile([128, 64], F16, tag="aux")
                        nc.tensor.transpose(
                            vps,
                            kvT[b2][64:128, 128 * jt : 128 * (jt + 1)],
                            ident_sb[64:128, :],
                        )
                        nc.vector.tensor_copy(vaug[b2][:, jt, 0:64], vps)

                pieces = [first]
                pieces += [mm_chunk(0, kc) for kc in range(1, 4)]
                pieces += [mm_chunk(1, kc) for kc in range(4)]
                pieces += [mm_chunk(2, kc) for kc in range(4)]
                pieces += [rope(qT01[b2], 128), rope(qT23[b2], 128),
                           rope(kvT[b2], 64), kv_tail]
                return pieces

            def emit_outproj_unit(b2, mt, copy_on_act=False):
                """One 128-token row block as 4 micro-pieces (one per 512-col
                chunk); the last piece issues the row block's y DMA."""
                ysl = []

                def piece(ncc):
                    def go():
                        if not ysl:
                            ysl.append(
                                y_pool.tile(
                                    [128, 4, 512], F16, tag="ys",
                                    name=f"ys_{b2}_{mt}",
                                )
                            )
                        ys = ysl[0]
                        ots = [OT0[b2], OT1[b2]]
                        yp = psA.tile([128, 512], F32, tag="aux")
                        for kq in range(2):
                            nc.tensor.matmul(
                                yp,
                                ots[kq][:, 128 * mt : 128 * (mt + 1)],
                                wo_sb[:, kq, 512 * ncc : 512 * (ncc + 1)],
                                start=(kq == 0),
                                stop=(kq == 1),
                            )
                        if copy_on_act:
                            nc.scalar.copy(ys[:, ncc, :], yp)
                        else:
                            nc.vector.tensor_copy(ys[:, ncc, :], yp)
                        if ncc == 3:
                            # y goes out on the ACT HWDGE queue so it never
                            # contends with the x-input stream on SP's queue
                            nc.scalar.dma_start(
                                out=y_d[
                                    T * b2 + 128 * mt : T * b2 + 128 * (mt + 1),
                                    :,
                                ],
                                in_=ys[:, :, :],
                            )
                    return go

                return [piece(ncc) for ncc in range(4)]

            def drain(b2, hp, par, v2, q, acc):
                # softmax denominators: 1/rowsum via exp(-ln(.)); ln/exp
                # share one ACT table set (natural_log_exp_and_others)
                lnr = lnr_pool.tile([1, 512], F32, tag="lnr")
                nc.scalar.activation(
                    out=lnr, in_=acc[64:65, :], func=AF.Ln
                )
                nc.scalar.activation(
                    out=lnr, in_=lnr, func=AF.Exp, scale=-1.0
                )
                scr = nc.dram_tensor(
                    f"scr_{b2}_{hp}_{par}_{v2}_{q}", [1, 512], F32
                )
                nc.gpsimd.dma_start(out=scr[:, :], in_=lnr)
                ibc = ibc_pool.tile([64, 512], F32, tag="ibc")
                nc.gpsimd.dma_start(
                    out=ibc,
                    in_=scr[:, :].partition_broadcast(64).squeeze(1),
                )
                ot = [OT0[b2], OT1[b2]][hp]
                tok = 1024 * v2 + 512 * q
                if par == 0:
                    nc.vector.tensor_mul(
                        ot[0:64, tok : tok + 512], acc[0:64, :], ibc
                    )
                else:
                    # DVE lanes are partition-locked; bounce through a
                    # partitions-0:64 tile and DMA into rows 64:128
                    on = tmp_pool.tile([64, 512], F16, tag="on")
                    nc.vector.tensor_mul(on, acc[0:64, :], ibc)
                    nc.gpsimd.dma_start(
                        out=ot[64:128, tok : tok + 512], in_=on
                    )

            def emit_attn_iter(b2, hp, par, v2, fillers):
                """Causal attention for one (batch, head, tq-half)."""
                qtile = [qT01[b2], qT23[b2]][hp]
                rows = slice(64 * par, 64 * par + 64)
                kstat = kvT[b2] if par == 0 else ktdup[b2]
                krows = slice(0, 64) if par == 0 else slice(64, 128)
                tq0 = 1024 * v2
                jmax = 8 * (v2 + 1)
                stop_j = (3, 7) if v2 == 0 else (11, 15)
                # tail k-blocks (W <= 512) share one score tile / one exp
                if v2 == 0:
                    jgroups = [[0], [1], [2], [3], [4, 5], [6, 7]]
                else:
                    jgroups = [[j] for j in range(12)]
                    jgroups += [[12, 13], [14, 15]]
                accs = {}
                jcount = 0
                for jg in jgroups:
                    sps = psS.tile([128, 1024], F32, tag="sps")
                    spans = []
                    for idx, j in enumerate(jg):
                        cb = 512 * idx
                        tqs = max(128 * j, tq0)
                        W = tq0 + 1024 - tqs
                        spans.append((j, cb, tqs, W))
                        ksl = slice(128 * j, 128 * (j + 1))
                        off = 0
                        while off < W:
                            nw = min(512, W - off)
                            qsl = slice(tqs + off, tqs + off + nw)
                            nc.tensor.matmul(
                                sps[:, cb + off : cb + off + nw],
                                kstat[krows, ksl], qtile[rows, qsl],
                                start=True, stop=True,
                            )
                            off += nw
                    es = es_pool.tile([128, 1024], F16, tag="es")
                    ext = spans[-1][1] + spans[-1][3]
                    nc.scalar.activation(
                        out=es[:, :ext], in_=sps[:, :ext],
                        func=AF.Exp, scale=SCALE,
                    )
                    for (j, cb, tqs, W) in spans:
                        if 128 * j >= tq0:
                            nc.vector.tensor_mul(
                                es[:, cb : cb + 128], es[:, cb : cb + 128],
                                dmask_sb,
                            )
                    for (j, cb, tqs, W) in spans:
                        off = 0
                        while off < W:
                            oc = tqs - tq0 + off
                            q = oc // 512
                            nw = min(512 - oc % 512, W - off)
                            if q not in accs:
                                accs[q] = psA.tile(
                                    [65, 512], F32, tag="acc",
                                    name=f"acc_{b2}_{hp}_{par}_{v2}_{q}",
                                )
                            nc.tensor.matmul(
                                accs[q][:, oc % 512 : oc % 512 + nw],
                                vaug[b2][:, j, :],
                                es[:, cb + off : cb + off + nw],
                                start=(j == 0),
                                stop=(j == stop_j[q]),
                            )
                            off += nw
                    for (j, cb, tqs, W) in spans:
                        for q in (0, 1):
                            if j == stop_j[q]:
                                drain(b2, hp, par, v2, q, accs[q])
                    jcount += len(jg)
                    if jcount > 1 and fillers:
                        fillers.pop(0)()
                        if jcount >= jmax // 2 and fillers:
                            fillers.pop(0)()
                        if len(jg) == 2 and fillers:
                            fillers.pop(0)()

            # ---------------- emission schedule ----------------
            def attn_window(iters, fillers):
                for (b2, hp, par, v2) in iters:
                    emit_attn_iter(b2, hp, par, v2, fillers)
                while fillers:
                    fillers.pop(0)()

            # QKV b0 groups 0-1 (enough for attn(b0, v2=0))
            for gl in range(2):
                for p in emit_qkv_group(0, gl):
                    p()

            heads0 = [(0, 0, 0, 0), (0, 0, 1, 0), (0, 1, 0, 0), (0, 1, 1, 0)]
            f = []
            for gl in range(2, 4):
                f += emit_qkv_group(0, gl)
            attn_window(heads0, f)

            heads1 = [(0, 0, 0, 1), (0, 0, 1, 1), (0, 1, 0, 1), (0, 1, 1, 1)]
            f = []
            for gl in range(2):
                f += emit_qkv_group(1, gl)
            attn_window(heads1, f)

            heads2 = [(1, 0, 0, 0), (1, 0, 1, 0), (1, 1, 0, 0), (1, 1, 1, 0)]
            f = []
            for gl in range(2, 4):
                f += emit_qkv_group(1, gl)
            for mt in range(0, 8):
                f += emit_outproj_unit(0, mt)
            attn_window(heads2, f)

            heads3 = [(1, 0, 0, 1), (1, 0, 1, 1), (1, 1, 0, 1), (1, 1, 1, 1)]
            f = []
            for mt in range(8, 16):
                f += emit_outproj_unit(0, mt)
            for mt in range(0, 8):
                f += emit_outproj_unit(1, mt)
            attn_window(heads3, f)

            for mt in range(8, 16):
                for p in emit_outproj_unit(1, mt, copy_on_act=True):
                    p()

    _split_waits(nc)
    return nc


def _host_inputs(x, sinusoidal_pos, Wq, bq, Wk, bk, Wv, bv, Wo):
    xT = np.ascontiguousarray(x.reshape(NTOK, C).T).astype(np.float16)

    sp = np.asarray(sinusoidal_pos, dtype=np.float32).reshape(T, D)
    cosd = np.repeat(sp[:, 0::2], 2, axis=1)     # [T, D]
    sind = np.repeat(sp[:, 1::2], 2, axis=1)
    cos2 = np.ascontiguousarray(
        np.concatenate([cosd.T, cosd.T], 0)).astype(np.float16)  # [128, T]
    sin2 = np.ascontiguousarray(
        np.concatenate([sind.T, sind.T], 0)).astype(np.float16)

    P = np.zeros((D, D), dtype=np.float32)
    P[: D // 2, D // 2 :] = np.eye(D // 2)
    P[D // 2 :, : D // 2] = -np.eye(D // 2)
    pmat = np.zeros((128, 128), dtype=np.float32)
    pmat[:64, :64] = P
    pmat[64:, 64:] = P
    pmat = pmat.astype(np.float16)

    f = np.arange(128)[None, :]
    p = np.arange(128)[:, None]
    dmask = (f >= p).astype(np.float16)          # S^T diag block: keep tk<=tq

    ident2 = np.concatenate([np.eye(64), np.eye(64)], 0).astype(np.float16)

    shared = {
        "xT": xT, "cos2": cos2, "sin2": sin2,
        "pmat": pmat, "dmask": dmask, "ident2": ident2,
        "vones": np.ones((128, B * NJ), dtype=np.float16),
    }
    per_core = []
    for c in range(8):
        # q head h uses kv head h % KVH (jnp.tile), so core c owns
        # q heads {c, c+8, c+16, c+24} and kv head c.
        heads = [c + KVH * g for g in range(HPC)]
        qrows = np.concatenate([np.arange(D * h, D * (h + 1)) for h in heads])
        Wq_c = Wq[qrows]
        Wk_c = Wk[D * c : D * (c + 1)]
        Wv_c = Wv[D * c : D * (c + 1)]
        wcatT = np.ascontiguousarray(
            np.concatenate([Wq_c, Wk_c, Wv_c], 0).T
        ).astype(np.float16)
        bcat = np.concatenate(
            [bq[qrows], bk[D * c : D * (c + 1)], bv[D * c : D * (c + 1)]]
        ).astype(np.float32)
        bqkv = np.ascontiguousarray(bcat.reshape(3, 128).T)
        woR = np.ascontiguousarray(Wo[:, qrows].T).astype(np.float16)
        per_core.append(dict(shared, wcatT=wcatT, bqkv=bqkv, woR=woR))
    return per_core


def kernel(x, mask, sinusoidal_pos, Wq, bq, Wk, bk, Wv, bv, Wo, bo):
    x = np.asarray(x, dtype=np.float32)
    in_maps = _host_inputs(
        x, sinusoidal_pos,
        np.asarray(Wq, np.float32), np.asarray(bq, np.float32),
        np.asarray(Wk, np.float32), np.asarray(bk, np.float32),
        np.asarray(Wv, np.float32), np.asarray(bv, np.float32),
        np.asarray(Wo, np.float32),
    )
    if "nc" not in _NC_CACHE:
        _NC_CACHE["nc"] = build_nc()
    res = run_bass_kernel_spmd(
        _NC_CACHE["nc"], in_maps, core_ids=list(range(8))
    )
    y = np.zeros((NTOK, C), dtype=np.float32)
    for r in res.results:
        y += r["y"].astype(np.float32)
    y += np.asarray(bo, np.float32)[None, :]
    return y.reshape(B, T, C)


# revision 29
# speedup vs baseline: 1.0430x; 1.0430x over previous
"""Trainium2 Bass kernel for GQA multi-head attention with RoPE.

Sharding: tensor-parallel over heads. Core c owns q-heads {c, c+8, c+16,
c+24} and kv-head c (q head h uses kv head h % 8). Each core computes its
QKV projection slice, RoPE, causal attention for its heads, and a partial
output projection (attn_out_local @ Wo[:, local].T) in fp16; the host sums
the 8 partial y's in fp32.

All matmul operands are fp16 (fp32/fp32r matmuls run in fp32_mode=HIGH at
less than half rate on TRN2). PSUM accumulation stays fp32.

Layouts (per core, per batch b):
  qT01/qT23 [128, T] roped q, two heads per tile (64 dims each)
  kvT       [128, T] rows 0:64 roped k, rows 64:128 v
  ktdup     [128, T] rows 64:128 = copy of roped k (odd heads read their
            stationary from partitions 64:128)
  scores    S^T [tk partitions, tq free]: exp(S^T) feeds P@V directly with
            v_aug = [v | ones] stationary; the ones row accumulates softmax
            row-sums in partition 64 of the PV accumulator.
  PV        accumulated per 512-token quarter ([65, 512] PSUM, 1 bank) so
            quarters drain (normalize into OT) while later k-blocks still
            accumulate the next quarter.
  y         [T*B, C] fp16 partial; host adds the 8 partials + bo.

Attention is processed one head per iteration (16 iterations of
(batch, head-pair, parity, tq-half)): per k-block the ACT engine does one
exp of [128, W] while the PE does W score columns + W PV columns, so the
two engines stay nearly balanced, and the PSUM budget (4 banks score
double-buffer + 2 banks PV quarters + 2 banks aux) leaves two slots for
QKV / output-projection work interleaved into the attention stream to keep
the PE dense (HAM re-throttles the PE clock to 1.2 GHz on idle gaps).
"""

import sys

sys.path.insert(0, "/opt/trn_rl_repo")

import numpy as np

import bass_rust
import concourse.bass as bass
import concourse.tile as tile
from concourse import mybir
from concourse.bass_utils import run_bass_kernel_spmd

B, T, C = 2, 2048, 2048
H, KVH, D = 32, 8, 64
NTOK = B * T                 # 4096
HPC = H // 8                 # 4 q heads per core
QL = HPC * D                 # 256 local q dims
KC = C // 128                # 16 contraction chunks
NJ = T // 128                # 16 k-blocks per batch
SCALE = float(D) ** -0.5

F32 = mybir.dt.float32
F16 = mybir.dt.float16
AF = mybir.ActivationFunctionType

_NC_CACHE = {}


def _split_waits(nc, limit=1):
    """Walrus in this toolchain allows only one sync-wait per instruction.

    Tile emits instructions with several sem waits (drain/barrier, phase
    boundaries). Hoist the excess onto same-engine NoOps inserted right
    before the instruction -- program order on the engine queue preserves
    the wait semantics.
    """
    ctr = 0
    for f in nc.m.functions:
        for blk in f.blocks:
            out = []
            changed = False
            for inst in list(blk.instructions):
                si = inst.sync_info
                if si is not None and len(si.on_wait) > limit:
                    waits = list(si.on_wait)
                    keep, excess = waits[:limit], waits[limit:]
                    for i in range(0, len(excess), limit):
                        ctr += 1
                        nop = mybir.InstNoOp(
                            name=f"I-wsplit-{ctr}", ins=[], outs=[]
                        )
                        nop.engine = inst.engine
                        nop.sync_info = bass_rust.SyncInfo(
                            on_wait=excess[i : i + limit], on_update=[]
                        )
                        out.append(nop)
                        changed = True
                    inst.sync_info = bass_rust.SyncInfo(
                        on_wait=keep, on_update=list(si.on_update)
                    )
                out.append(inst)
            if changed:
                blk.instructions = out
    return ctr


def build_nc():
    nc = bass.Bass(trn_type="TRN2")

    xT_d = nc.dram_tensor("xT", [C, NTOK], F16, kind="ExternalInput")
    wcat_d = nc.dram_tensor("wcatT", [C, 384], F16, kind="ExternalInput")
    bias_d = nc.dram_tensor("bqkv", [128, 3], F32, kind="ExternalInput")
    wo_d = nc.dram_tensor("woR", [QL, C], F16, kind="ExternalInput")
    cos_d = nc.dram_tensor("cos2", [128, T], F16, kind="ExternalInput")
    sin_d = nc.dram_tensor("sin2", [128, T], F16, kind="ExternalInput")
    pmat_d = nc.dram_tensor("pmat", [128, 128], F16, kind="ExternalInput")
    dmask_d = nc.dram_tensor("dmask", [128, 128], F16, kind="ExternalInput")
    ident_d = nc.dram_tensor("ident2", [128, 64], F16, kind="ExternalInput")
    vones_d = nc.dram_tensor("vones", [128, B * NJ], F16, kind="ExternalInput")
    y_d = nc.dram_tensor("y", [NTOK, C], F16, kind="ExternalOutput")

    xT_v = xT_d.rearrange("(k p) t -> p k t", p=128)

    with tile.TileContext(nc) as tc:
        with (
            tc.tile_pool(name="consts", bufs=1) as consts,
            tc.tile_pool(name="acts", bufs=1) as acts,
            tc.tile_pool(name="xs", bufs=2) as xs_pool,
            tc.tile_pool(name="tmp", bufs=2) as tmp_pool,
            tc.tile_pool(name="es", bufs=6) as es_pool,
            tc.tile_pool(name="lnr", bufs=4) as lnr_pool,
            tc.tile_pool(name="ibc", bufs=4) as ibc_pool,
            tc.tile_pool(name="ystage", bufs=3) as y_pool,
            tc.tile_pool(name="psS", bufs=2, space="PSUM") as psS,
            tc.tile_pool(name="psA", bufs=2, space="PSUM") as psA,
        ):
            # ---- constants (ident/cos first: the HAM warm-up needs them;
            # wcat k0-3 + the first x quarter unblock the first QKV chunk) ----
            ident_sb = consts.tile([128, 64], F16, tag="ident")
            nc.sync.dma_start(out=ident_sb, in_=ident_d[:, :])
            cos_sb = consts.tile([128, T], F16, tag="cos")
            nc.sync.dma_start(out=cos_sb, in_=cos_d[:, :])
            wcat_sb = consts.tile([128, KC, 384], F16, tag="wcat")
            for k in range(4):
                nc.sync.dma_start(
                    out=wcat_sb[:, k, :], in_=wcat_d[128 * k : 128 * (k + 1), :]
                )

            # HAM warm-up: dummy matmuls so the PE clock gate is at 8/8
            # (needs ~3.4us sustained busy) while the input DMAs stream
            wps = psA.tile([128, 512], F32, tag="aux")
            for i in range(32):
                nc.tensor.matmul(
                    wps[0:64, :],
                    ident_sb[0:64, :],
                    cos_sb[0:64, 0:512],
                    start=(i == 0),
                    stop=(i == 31),
                )
            wsink = consts.tile([64, 512], F16, tag="wsink")
            nc.vector.tensor_copy(wsink, wps[0:64, :])
            nc.gpsimd.dma_start(
                out=nc.dram_tensor("wsink_d", [64, 512], F16)[:, :],
                in_=wsink,
            )

            for k in range(4, KC):
                nc.sync.dma_start(
                    out=wcat_sb[:, k, :], in_=wcat_d[128 * k : 128 * (k + 1), :]
                )
            bias_sb = consts.tile([128, 3], F32, tag="bias")
            nc.scalar.dma_start(out=bias_sb, in_=bias_d[:, :])
            sin_sb = consts.tile([128, T], F16, tag="sin")
            nc.scalar.dma_start(out=sin_sb, in_=sin_d[:, :])
            pmat_sb = consts.tile([128, 128], F16, tag="pmat")
            nc.scalar.dma_start(out=pmat_sb, in_=pmat_d[:, :])
            dmask_sb = consts.tile([128, 128], F16, tag="dmask")
            nc.scalar.dma_start(out=dmask_sb, in_=dmask_d[:, :])
            wo_sb = consts.tile([128, 2, C], F16, tag="wo")
            for k in range(2):
                nc.gpsimd.dma_start(
                    out=wo_sb[:, k, :], in_=wo_d[128 * k : 128 * (k + 1), :]
                )

            def _mk(nm, shape, dt):
                return [
                    acts.tile(shape, dt, tag=f"{nm}_{b}", name=f"{nm}_{b}")
                    for b in range(B)
                ]

            qT01 = _mk("qT01", [128, T], F16)
            qT23 = _mk("qT23", [128, T], F16)
            kvT = _mk("kvT", [128, T], F16)
            ktdup = _mk("ktd", [128, T], F16)
            vaug = _mk("vaug", [128, NJ, 65], F16)
            OT0 = _mk("OT0", [128, T], F16)
            OT1 = _mk("OT1", [128, T], F16)
            for b in range(B):
                nc.gpsimd.dma_start(
                    out=vaug[b][:, :, 64:65],
                    in_=vones_d[:, NJ * b : NJ * (b + 1)].unsqueeze(2),
                )

            qkv_dst = [[qT01[b], qT23[b], kvT[b]] for b in range(B)]

            def emit_qkv_group(b2, gl):
                """QKV projection + RoPE + k-dup + v-transpose for 512 tokens.

                Returns emitter thunks (micro-pieces of <=1us of PE work) so
                the caller can weave them into the attention stream without
                stalling the exp chain.
                """
                base = 512 * gl
                gbase = T * b2 + base
                xts = []
                pss = {}

                def load_x():
                    # four DMAs so the first contraction chunk unblocks early
                    xt = xs_pool.tile([128, KC, 512], F16, tag="xs")
                    for kq in range(4):
                        nc.sync.dma_start(
                            out=xt[:, 4 * kq : 4 * kq + 4, :],
                            in_=xT_v[:, 4 * kq : 4 * kq + 4, gbase : gbase + 512],
                        )
                    xts.append(xt)

                def mm_chunk(m, kc):
                    # 4 of the 16 contraction steps of pass m
                    def go():
                        if m not in pss:
                            pss[m] = psA.tile(
                                [128, 512], F32, tag="aux",
                                name=f"qkvps_{b2}_{gl}_{m}",
                            )
                        ps = pss[m]
                        for k in range(4 * kc, 4 * kc + 4):
                            nc.tensor.matmul(
                                ps,
                                wcat_sb[:, k, 128 * m : 128 * (m + 1)],
                                xts[0][:, k, :],
                                start=(k == 0),
                                stop=(k == KC - 1),
                            )
                        if kc == 3:
                            nc.vector.tensor_scalar_add(
                                out=qkv_dst[b2][m][:, base : base + 512],
                                in0=ps,
                                scalar1=bias_sb[:, m : m + 1],
                            )
                    return go

                def first():
                    load_x()
                    mm_chunk(0, 0)()

                def rope(dst, rn):
                    def go():
                        rot = psA.tile([128, 512], F32, tag="aux")
                        nc.tensor.matmul(
                            rot[:rn, :],
                            pmat_sb[:rn, :rn],
                            dst[:rn, base : base + 512],
                            start=True,
                            stop=True,
                        )
                        tmp = tmp_pool.tile([128, 512], F16, tag="tmp")
                        nc.vector.tensor_mul(
                            tmp[:rn, :], rot[:rn, :],
                            sin_sb[:rn, base : base + 512],
                        )
                        nc.vector.tensor_mul(
                            dst[:rn, base : base + 512],
                            dst[:rn, base : base + 512],
                            cos_sb[:rn, base : base + 512],
                        )
                        nc.vector.tensor_add(
                            dst[:rn, base : base + 512],
                            dst[:rn, base : base + 512],
                            tmp[:rn, :],
                        )
                    return go

                def kv_tail():
                    # duplicate roped k into ktdup partitions 64:128
                    nc.gpsimd.dma_start(
                        out=ktdup[b2][64:128, base : base + 512],
                        in_=kvT[b2][0:64, base : base + 512],
                    )
                    # v transposes (v is not roped)
                    for jj in range(4):
                        jt = 4 * gl + jj
                        vps = psA.tile([128, 64], F16, tag="aux")
                        nc.tensor.transpose(
                            vps,
                            kvT[b2][64:128, 128 * jt : 128 * (jt + 1)],
                            ident_sb[64:128, :],
                        )
                        nc.vector.tensor_copy(vaug[b2][:, jt, 0:64], vps)

                pieces = [first]
                pieces += [mm_chunk(0, kc) for kc in range(1, 4)]
                pieces += [mm_chunk(1, kc) for kc in range(4)]
                pieces += [mm_chunk(2, kc) for kc in range(4)]
                pieces += [rope(qT01[b2], 128), rope(qT23[b2], 128),
                           rope(kvT[b2], 64), kv_tail]
                return pieces

            def emit_outproj_unit(b2, mt, copy_on_act=False):
                """One 128-token row block as 4 micro-pieces (one per 512-col
                chunk); the last piece issues the row block's y DMA."""
                ysl = []

                def piece(ncc):
                    def go():
                        if not ysl:
                            ysl.append(
                                y_pool.tile(
                                    [128, 4, 512], F16, tag="ys",
                                    name=f"ys_{b2}_{mt}",
                                )
                            )
                        ys = ysl[0]
                        ots = [OT0[b2], OT1[b2]]
                        yp = psA.tile([128, 512], F32, tag="aux")
                        for kq in range(2):
                            nc.tensor.matmul(
                                yp,
                                ots[kq][:, 128 * mt : 128 * (mt + 1)],
                                wo_sb[:, kq, 512 * ncc : 512 * (ncc + 1)],
                                start=(kq == 0),
                                stop=(kq == 1),
                            )
                        if copy_on_act:
                            nc.scalar.copy(ys[:, ncc, :], yp)
                        else:
                            nc.vector.tensor_copy(ys[:, ncc, :], yp)
                        if ncc == 3:
                            nc.sync.dma_start(
                                out=y_d[
                                    T * b2 + 128 * mt : T * b2 + 128 * (mt + 1),
                                    :,
                                ],
                                in_=ys[:, :, :],
                            )
                    return go

                return [piece(ncc) for ncc in range(4)]

            def drain(b2, hp, par, v2, q, acc):
                # softmax denominators: 1/rowsum via exp(-ln(.)); ln/exp
                # share one ACT table set (natural_log_exp_and_others)
                lnr = lnr_pool.tile([1, 512], F32, tag="lnr")
                nc.scalar.activation(
                    out=lnr, in_=acc[64:65, :], func=AF.Ln
                )
                nc.scalar.activation(
                    out=lnr, in_=lnr, func=AF.Exp, scale=-1.0
                )
                scr = nc.dram_tensor(
                    f"scr_{b2}_{hp}_{par}_{v2}_{q}", [1, 512], F32
                )
                nc.gpsimd.dma_start(out=scr[:, :], in_=lnr)
                ibc = ibc_pool.tile([64, 512], F32, tag="ibc")
                nc.gpsimd.dma_start(
                    out=ibc,
                    in_=scr[:, :].partition_broadcast(64).squeeze(1),
                )
                ot = [OT0[b2], OT1[b2]][hp]
                tok = 1024 * v2 + 512 * q
                if par == 0:
                    nc.vector.tensor_mul(
                        ot[0:64, tok : tok + 512], acc[0:64, :], ibc
                    )
                else:
                    # DVE lanes are partition-locked; bounce through a
                    # partitions-0:64 tile and DMA into rows 64:128
                    on = tmp_pool.tile([64, 512], F16, tag="on")
                    nc.vector.tensor_mul(on, acc[0:64, :], ibc)
                    nc.gpsimd.dma_start(
                        out=ot[64:128, tok : tok + 512], in_=on
                    )

            def emit_attn_iter(b2, hp, par, v2, fillers):
                """Causal attention for one (batch, head, tq-half)."""
                qtile = [qT01[b2], qT23[b2]][hp]
                rows = slice(64 * par, 64 * par + 64)
                kstat = kvT[b2] if par == 0 else ktdup[b2]
                krows = slice(0, 64) if par == 0 else slice(64, 128)
                tq0 = 1024 * v2
                jmax = 8 * (v2 + 1)
                stop_j = (3, 7) if v2 == 0 else (11, 15)
                # tail k-blocks (W <= 512) share one score tile / one exp
                if v2 == 0:
                    jgroups = [[0], [1], [2], [3], [4, 5], [6, 7]]
                else:
                    jgroups = [[j] for j in range(12)]
                    jgroups += [[12, 13], [14, 15]]
                accs = {}
                jcount = 0
                for jg in jgroups:
                    sps = psS.tile([128, 1024], F32, tag="sps")
                    spans = []
                    for idx, j in enumerate(jg):
                        cb = 512 * idx
                        tqs = max(128 * j, tq0)
                        W = tq0 + 1024 - tqs
                        spans.append((j, cb, tqs, W))
                        ksl = slice(128 * j, 128 * (j + 1))
                        off = 0
                        while off < W:
                            nw = min(512, W - off)
                            qsl = slice(tqs + off, tqs + off + nw)
                            nc.tensor.matmul(
                                sps[:, cb + off : cb + off + nw],
                                kstat[krows, ksl], qtile[rows, qsl],
                                start=True, stop=True,
                            )
                            off += nw
                    es = es_pool.tile([128, 1024], F16, tag="es")
                    ext = spans[-1][1] + spans[-1][3]
                    nc.scalar.activation(
                        out=es[:, :ext], in_=sps[:, :ext],
                        func=AF.Exp, scale=SCALE,
                    )
                    for (j, cb, tqs, W) in spans:
                        if 128 * j >= tq0:
                            nc.vector.tensor_mul(
                                es[:, cb : cb + 128], es[:, cb : cb + 128],
                                dmask_sb,
                            )
                    for (j, cb, tqs, W) in spans:
                        off = 0
                        while off < W:
                            oc = tqs - tq0 + off
                            q = oc // 512
                            nw = min(512 - oc % 512, W - off)
                            if q not in accs:
                                accs[q] = psA.tile(
                                    [65, 512], F32, tag="acc",
                                    name=f"acc_{b2}_{hp}_{par}_{v2}_{q}",
                                )
                            nc.tensor.matmul(
                                accs[q][:, oc % 512 : oc % 512 + nw],
                                vaug[b2][:, j, :],
                                es[:, cb + off : cb + off + nw],
                                start=(j == 0),
                                stop=(j == stop_j[q]),
                            )
                            off += nw
                    for (j, cb, tqs, W) in spans:
                        for q in (0, 1):
                            if j == stop_j[q]:
                                drain(b2, hp, par, v2, q, accs[q])
                    jcount += len(jg)
                    if jcount > 1 and fillers:
                        fillers.pop(0)()
                        if jcount >= jmax // 2 and fillers:
                            fillers.pop(0)()
                        if len(jg) == 2 and fillers:
                            fillers.pop(0)()

            # ---------------- emission schedule ----------------
            def attn_window(iters, fillers):
                for (b2, hp, par, v2) in iters:
                    emit_attn_iter(b2, hp, par, v2, fillers)
                while fillers:
                    fillers.pop(0)()

            # QKV b0 groups 0-1 (enough for attn(b0, v2=0))
            for gl in range(2):
                for p in emit_qkv_group(0, gl):
                    p()

            heads0 = [(0, 0, 0, 0), (0, 0, 1, 0), (0, 1, 0, 0), (0, 1, 1, 0)]
            f = []
            for gl in range(2, 4):
                f += emit_qkv_group(0, gl)
            attn_window(heads0, f)

            heads1 = [(0, 0, 0, 1), (0, 0, 1, 1), (0, 1, 0, 1), (0, 1, 1, 1)]
            f = []
            for gl in range(2):
                f += emit_qkv_group(1, gl)
            attn_window(heads1, f)

            heads2 = [(1, 0, 0, 0), (1, 0, 1, 0), (1, 1, 0, 0), (1, 1, 1, 0)]
            f = []
            for gl in range(2, 4):
                f += emit_qkv_group(1, gl)
            for mt in range(0, 8):
                f += emit_outproj_unit(0, mt)
            attn_window(heads2, f)

            heads3 = [(1, 0, 0, 1), (1, 0, 1, 1), (1, 1, 0, 1), (1, 1, 1, 1)]
            f = []
            for mt in range(8, 16):
                f += emit_outproj_unit(0, mt)
            for mt in range(0, 8):
                f += emit_outproj_unit(1, mt)
            attn_window(heads3, f)

            for mt in range(8, 16):
                for p in emit_outproj_unit(1, mt, copy_on_act=True):
                    p()

    _split_waits(nc)
    return nc


def _host_inputs(x, sinusoidal_pos, Wq, bq, Wk, bk, Wv, bv, Wo):
    xT = np.ascontiguousarray(x.reshape(NTOK, C).T).astype(np.float16)

    sp = np.asarray(sinusoidal_pos, dtype=np.float32).reshape(T, D)
    cosd = np.repeat(sp[:, 0::2], 2, axis=1)     # [T, D]
    sind = np.repeat(sp[:, 1::2], 2, axis=1)
    cos2 = np.ascontiguousarray(
        np.concatenate([cosd.T, cosd.T], 0)).astype(np.float16)  # [128, T]
    sin2 = np.ascontiguousarray(
        np.concatenate([sind.T, sind.T], 0)).astype(np.float16)

    P = np.zeros((D, D), dtype=np.float32)
    P[: D // 2, D // 2 :] = np.eye(D // 2)
    P[D // 2 :, : D // 2] = -np.eye(D // 2)
    pmat = np.zeros((128, 128), dtype=np.float32)
    pmat[:64, :64] = P
    pmat[64:, 64:] = P
    pmat = pmat.astype(np.float16)

    f = np.arange(128)[None, :]
    p = np.arange(128)[:, None]
    dmask = (f >= p).astype(np.float16)          # S^T diag block: keep tk<=tq

    ident2 = np.concatenate([np.eye(64), np.eye(64)], 0).astype(np.float16)

    shared = {
        "xT": xT, "cos2": cos2, "sin2": sin2,
        "pmat": pmat, "dmask": dmask, "ident2": ident2,
        "vones": np.ones((128, B * NJ), dtype=np.float16),
    }
    per_core = []
    for c in range(8):
        # q head h uses kv head h % KVH (jnp.tile), so core c owns
        # q heads {c, c+8, c+16, c+24} and kv head c.
        heads = [c + KVH * g for g in range(HPC)]
        qrows = np.concatenate([np.arange(D * h, D * (h + 1)) for h in heads])
        Wq_c = Wq[qrows]
        Wk_c = Wk[D * c : D * (c + 1)]
        Wv_c = Wv[D * c : D * (c + 1)]
        wcatT = np.ascontiguousarray(
            np.concatenate([Wq_c, Wk_c, Wv_c], 0).T
        ).astype(np.float16)
        bcat = np.concatenate(
            [bq[qrows], bk[D * c : D * (c + 1)], bv[D * c : D * (c + 1)]]
        ).astype(np.float32)
        bqkv = np.ascontiguousarray(bcat.reshape(3, 128).T)
        woR = np.ascontiguousarray(Wo[:, qrows].T).astype(np.float16)
        per_core.append(dict(shared, wcatT=wcatT, bqkv=bqkv, woR=woR))
    return per_core


def kernel(x, mask, sinusoidal_pos, Wq, bq, Wk, bk, Wv, bv, Wo, bo):
    x = np.asarray(x, dtype=np.float32)
    in_maps = _host_inputs(
        x, sinusoidal_pos,
        np.asarray(Wq, np.float32), np.asarray(bq, np.float32),
        np.asarray(Wk, np.float32), np.asarray(bk, np.float32),
        np.asarray(Wv, np.float32), np.asarray(bv, np.float32),
        np.asarray(Wo, np.float32),
    )
    if "nc" not in _NC_CACHE:
        _NC_CACHE["nc"] = build_nc()
    res = run_bass_kernel_spmd(
        _NC_CACHE["nc"], in_maps, core_ids=list(range(8))
    )
    y = np.zeros((NTOK, C), dtype=np.float32)
    for r in res.results:
        y += r["y"].astype(np.float32)
    y += np.asarray(bo, np.float32)[None, :]
    return y.reshape(B, T, C)
